# revision 1
# baseline (speedup 1.0000x reference)
"""Trainium2 Bass kernel for the controlled-unitary problem.

reference semantics (control=0, num_qubits=13, dim=8192):
    mask bit = 1 << 12, so columns/rows with that bit set are idx 4096..8191.
    out[:, c0] = state[:, c0]                       (control bit off: untouched)
    out[:, c1] = state[:, c1] @ target[c1, c1]      (controlled unitary)

Device work: complex [256,4096] @ [4096,4096] GEMM = 4 real GEMMs.
Sharding: output columns of the GEMM split 8 ways (each core gets a
[4096, 512] slab of the target block; every weight byte moves once).

Per-core kernel (v2):
  - A planes a_r, a_i and a_n = -a_i (negation host-side) let the real
    part accumulate directly in PSUM: bank_r += a_r.b_r + a_n.b_i,
    bank_i += a_r.b_i + a_i.b_r  ->  4 PSUM banks (2 M-tiles x re/im),
    combine is just a PSUM->SBUF copy.
  - DMA on both HWDGE rings: A planes + outputs on nc.sync (SP ring),
    B planes on nc.scalar (ACT ring).
  - K streamed in ramped chunks (small first chunk so the PE starts
    early, big later chunks for DMA efficiency).
"""

import os

import numpy as np

BATCH = 256
DIM = 8192
HALF = 4096
N_CORES = 8
NSH = HALF // N_CORES  # 512 output columns per core
KT = HALF // 128  # 32 k-tiles
MT = BATCH // 128  # 2 m-tiles
CHUNKS = [1, 1, 2, 4, 8, 8, 8]  # k-tiles per DMA chunk (sums to KT)
CHMAX = max(CHUNKS)

# matmul dtype: "float32r" = full-rate fp32 path, "float16" = half traffic
DT_NAME = os.environ.get("KERNEL_DT", "float16")

_CACHE = {}


def _np_dtype(dt_name):
    return np.float16 if dt_name == "float16" else np.float32


def _build(dt_name):
    import concourse.mybir as mybir
    import concourse.tile as tile
    from concourse import bacc

    DT = getattr(mybir.dt, dt_name)
    F32 = mybir.dt.float32

    nc = bacc.Bacc("TRN2", target_bir_lowering=False, debug=False,
                   num_devices=N_CORES)

    a_r = nc.dram_tensor("a_r", [128, KT, BATCH], DT, kind="ExternalInput")
    a_i = nc.dram_tensor("a_i", [128, KT, BATCH], DT, kind="ExternalInput")
    b_r = nc.dram_tensor("b_r", [128, KT, NSH], DT, kind="ExternalInput")
    b_i = nc.dram_tensor("b_i", [128, KT, NSH], DT, kind="ExternalInput")
    c_r = nc.dram_tensor("c_r", [BATCH, NSH], F32, kind="ExternalOutput")
    c_i = nc.dram_tensor("c_i", [BATCH, NSH], F32, kind="ExternalOutput")

    with tile.TileContext(nc) as tc:
        with (
            tc.tile_pool(name="ap", bufs=4) as ap_pool,
            tc.tile_pool(name="bp", bufs=4) as bp_pool,
            tc.tile_pool(name="op", bufs=2) as o_pool,
            tc.tile_pool(name="ps", bufs=1, space="PSUM") as ps_pool,
        ):
            # Gauss 3-multiplication complex GEMM:
            #   k1 = (a_r+a_i).b_r   k2 = a_r.(b_i-b_r)   k3n = (-a_i).(b_r+b_i)
            #   C_r = k1 + k3n       C_i = k1 + k2
            ps = {}
            for m in range(MT):
                for comp in ("k1", "k2", "k3"):
                    ps[(m, comp)] = ps_pool.tile(
                        [128, NSH], F32, name=f"ps_{m}_{comp}"
                    )

            k0 = 0
            for ch in CHUNKS:
                nb = 3 if ch == CHMAX else 2
                ar_t = ap_pool.tile([128, ch, BATCH], DT, name=f"ar{ch}", bufs=nb)
                ai_t = ap_pool.tile([128, ch, BATCH], DT, name=f"ai{ch}", bufs=nb)
                as_t = ap_pool.tile([128, ch, BATCH], DT, name=f"as{ch}", bufs=nb)
                br_t = bp_pool.tile([128, ch, NSH], DT, name=f"br{ch}", bufs=nb)
                bi_t = bp_pool.tile([128, ch, NSH], DT, name=f"bi{ch}", bufs=nb)
                bs_t = bp_pool.tile([128, ch, NSH], DT, name=f"bs{ch}", bufs=nb)
                ksl = slice(k0, k0 + ch)
                # two HWDGE rings, balanced: SP ring gets a_r + b_r,
                # ACT ring gets a_i + b_i (6.3MB each)
                nc.sync.dma_start(ar_t[:], a_r[:, ksl, :])
                nc.scalar.dma_start(ai_t[:], a_i[:, ksl, :])
                nc.sync.dma_start(br_t[:], b_r[:, ksl, :])
                nc.scalar.dma_start(bi_t[:], b_i[:, ksl, :])
                # DVE operand prep (fp16 SBUF 2x/4x modes, overlapped with PE):
                #   as = a_r + a_i;  ai <- -a_i (in place, becomes a_n)
                #   bs = b_r + b_i;  bi <- b_i - b_r (in place, becomes b_d)
                nc.vector.tensor_tensor(as_t[:], ar_t[:], ai_t[:],
                                        mybir.AluOpType.add)
                nc.vector.tensor_scalar_mul(ai_t[:], ai_t[:], -1.0)
                nc.vector.tensor_tensor(bs_t[:], br_t[:], bi_t[:],
                                        mybir.AluOpType.add)
                nc.vector.tensor_tensor(bi_t[:], bi_t[:], br_t[:],
                                        mybir.AluOpType.subtract)
                last_chunk = k0 + ch == KT
                # product-major order inside the chunk: k1 matmuls only
                # depend on the `as` prep, so the PE starts them while DVE
                # still computes bs/bd for k2/k3
                operands = {
                    "k1": (as_t, br_t),
                    "k2": (ar_t, bi_t),
                    "k3": (ai_t, bs_t),
                }
                for comp in ("k1", "k2", "k3"):
                    lhs_t, rhs_t = operands[comp]
                    for m in range(MT):
                        for kk in range(ch):
                            k = k0 + kk
                            msl = slice(m * 128, (m + 1) * 128)
                            nc.tensor.matmul(
                                ps[(m, comp)][:], lhs_t[:, kk, msl],
                                rhs_t[:, kk, :], start=(k == 0),
                                stop=(last_chunk and kk == ch - 1),
                            )
                k0 += ch

            for m in range(MT):
                msl = slice(m * 128, (m + 1) * 128)
                t2 = o_pool.tile([128, NSH], F32, name="t2")
                t3 = o_pool.tile([128, NSH], F32, name="t3")
                out_r = o_pool.tile([128, NSH], F32, name="out_r")
                out_i = o_pool.tile([128, NSH], F32, name="out_i")
                nc.vector.tensor_copy(t3[:], ps[(m, "k3")][:])
                nc.vector.tensor_copy(t2[:], ps[(m, "k2")][:])
                nc.vector.tensor_tensor(out_r[:], ps[(m, "k1")][:], t3[:],
                                        mybir.AluOpType.add)
                nc.vector.tensor_tensor(out_i[:], ps[(m, "k1")][:], t2[:],
                                        mybir.AluOpType.add)
                nc.sync.dma_start(c_r[msl, :], out_r[:])
                nc.scalar.dma_start(c_i[msl, :], out_i[:])

    nc.compile()
    return nc


def _get_nc(dt_name):
    if dt_name not in _CACHE:
        _CACHE[dt_name] = _build(dt_name)
    return _CACHE[dt_name]


def _pack_kxm(mat_t, np_dt):
    # mat_t: [4096, F] (k-major) -> [128, KT, F] with k = kt*128 + p
    f = mat_t.shape[1]
    return np.ascontiguousarray(
        mat_t.reshape(KT, 128, f).transpose(1, 0, 2).astype(np_dt)
    )


def run_device(A, B, dt_name=DT_NAME, trace=False):
    """A: [256, 4096] complex64, B: [4096, 4096] complex64.
    Returns C = A @ B as [256, 4096] complex64 plus the raw results."""
    from concourse import bass_utils

    nc = _get_nc(dt_name)
    np_dt = _np_dtype(dt_name)

    at = A.T  # [4096, 256]
    a_r = _pack_kxm(np.ascontiguousarray(at.real), np_dt)
    a_i = _pack_kxm(np.ascontiguousarray(at.imag), np_dt)
    br_full = B.real
    bi_full = B.imag

    in_maps = []
    for c in range(N_CORES):
        csl = slice(c * NSH, (c + 1) * NSH)
        in_maps.append({
            "a_r": a_r,
            "a_i": a_i,
            "b_r": _pack_kxm(np.ascontiguousarray(br_full[:, csl]), np_dt),
            "b_i": _pack_kxm(np.ascontiguousarray(bi_full[:, csl]), np_dt),
        })

    res = bass_utils.run_bass_kernel_spmd(
        nc, in_maps, core_ids=list(range(N_CORES)), trace=trace
    )

    out = np.empty((BATCH, HALF), dtype=np.complex64)
    for c in range(N_CORES):
        csl = slice(c * NSH, (c + 1) * NSH)
        out.real[:, csl] = res.results[c]["c_r"]
        out.imag[:, csl] = res.results[c]["c_i"]
    return out, res


def kernel(state, target_matrix, control, num_qubits):
    state = np.asarray(state)
    target_matrix = np.asarray(target_matrix)
    control = int(control)
    num_qubits = int(num_qubits)
    dim = 1 << num_qubits

    assert state.shape == (BATCH, DIM) and dim == DIM, (
        "kernel hardcoded for [256, 8192]"
    )

    mask = 1 << (num_qubits - control - 1)
    idx = np.arange(dim)
    c1 = idx[(idx & mask) != 0]  # columns with control bit set

    if control == 0:
        A = state[:, HALF:]
        B = target_matrix[HALF:, HALF:]
    else:
        A = state[:, c1]
        B = target_matrix[np.ix_(c1, c1)]
    A = np.ascontiguousarray(A, dtype=np.complex64)
    B = np.ascontiguousarray(B, dtype=np.complex64)

    C, _ = run_device(A, B)

    out = state.astype(np.complex64, copy=True)
    out[:, c1] = C
    return out



# revision 2
# speedup vs baseline: 1.3603x; 1.3603x over previous
"""Trainium2 Bass kernel for the controlled-unitary problem.

reference semantics (control=0, num_qubits=13, dim=8192):
    mask bit = 1 << 12, so columns/rows with that bit set are idx 4096..8191.
    out[:, c0] = state[:, c0]                       (control bit off: untouched)
    out[:, c1] = state[:, c1] @ target[c1, c1]      (controlled unitary)

Device work: complex [256,4096] @ [4096,4096] GEMM, Gauss 3-mult.
Sharding: output columns split 8 ways ([4096, 512] slab per core).

v3 design (per core):
  - Host sends planes ar, an=-ai, bd=bi-br, bs=br+bi packed per k-tile
    into TWO DRAM tensors (even/odd k-tiles), one per HWDGE ring.
    Row layout per partition: [ar(256) | an(256) | bd(512) | bs(512)].
  - Products: k2 = ar.bd and k3n = an.bs need NO device prep;
    j1 = (ar-an).(bs-bd) = 2*k1 needs two plain DVE subtracts per step.
    Combine: Cr = 0.5*j1 + k3n, Ci = 0.5*j1 + k2 (scale folded into the
    ACT PSUM->SBUF copy).
  - One big SBUF tile per ring (subtile deps), DMA in 9 ramped steps per
    ring, everything live (no slot recycling), outputs in fp16.
  - Last step emits m0's matmuls first so m0's combine + store overlap
    m1's final matmuls.
"""

import os

import numpy as np

BATCH = 256
DIM = 8192
HALF = 4096
N_CORES = 8
NSH = HALF // N_CORES  # 512 output columns per core
KT = HALF // 128  # 32 k-tiles
MT = BATCH // 128  # 2 m-tiles
KT_R = KT // 2  # 16 k-tiles per ring
# per-ring DMA step sizes (k-tiles per dma_start); small first steps so
# the PE starts early, 2-tile steps later for DMA efficiency
STEPS = [1, 1, 2, 2, 2, 2, 2, 2, 2]
assert sum(STEPS) == KT_R
ROW = 2 * BATCH + 2 * NSH  # 1536 packed elements per k-tile per partition

DT_NAME = "float16"  # kept for test.py compat

_CACHE = {}


def _build(dt_name="float16"):
    import concourse.mybir as mybir
    import concourse.tile as tile
    from concourse import bacc

    DT = mybir.dt.float16
    F32 = mybir.dt.float32

    nc = bacc.Bacc("TRN2", target_bir_lowering=False, debug=False,
                   num_devices=N_CORES)

    in0 = nc.dram_tensor("in0", [128, KT_R, ROW], DT, kind="ExternalInput")
    in1 = nc.dram_tensor("in1", [128, KT_R, ROW], DT, kind="ExternalInput")
    c_r = nc.dram_tensor("c_r", [BATCH, NSH], DT, kind="ExternalOutput")
    c_i = nc.dram_tensor("c_i", [BATCH, NSH], DT, kind="ExternalOutput")

    # packed row segment offsets
    AR0 = 0
    AN0 = BATCH
    BD0 = 2 * BATCH
    BS0 = 2 * BATCH + NSH

    with tile.TileContext(nc) as tc:
        with (
            tc.tile_pool(name="inp", bufs=1) as in_pool,
            tc.tile_pool(name="prep", bufs=1) as pr_pool,
            tc.tile_pool(name="op", bufs=1) as o_pool,
            tc.tile_pool(name="ps", bufs=1, space="PSUM") as ps_pool,
        ):
            it = [
                in_pool.tile([128, KT_R, ROW], DT, name=f"it{r}")
                for r in range(2)
            ]
            as_t = [
                pr_pool.tile([128, KT_R, BATCH], DT, name=f"as{r}")
                for r in range(2)
            ]
            bsub_t = [
                pr_pool.tile([128, KT_R, NSH], DT, name=f"bsub{r}")
                for r in range(2)
            ]
            ps = {}
            for m in range(MT):
                for comp in ("j1", "k2", "k3"):
                    ps[(m, comp)] = ps_pool.tile(
                        [128, NSH], F32, name=f"ps_{m}_{comp}"
                    )

            dram = (in0, in1)
            ring = (nc.sync, nc.scalar)

            def emit_mms(kt, m, comps=("k2", "k3", "j1")):
                r = kt % 2
                pos = kt // 2
                msl = {  # lhsT slice per product
                    "k2": slice(AR0 + m * 128, AR0 + (m + 1) * 128),
                    "k3": slice(AN0 + m * 128, AN0 + (m + 1) * 128),
                }
                rhs = {
                    "k2": it[r][:, pos, BD0:BD0 + NSH],
                    "k3": it[r][:, pos, BS0:BS0 + NSH],
                    "j1": bsub_t[r][:, pos, :],
                }
                for comp in comps:
                    if comp == "j1":
                        lhsT = as_t[r][:, pos, m * 128:(m + 1) * 128]
                    else:
                        lhsT = it[r][:, pos, msl[comp]]
                    nc.tensor.matmul(
                        ps[(m, comp)][:], lhsT, rhs[comp],
                        start=(kt == 0), stop=(kt == KT - 1),
                    )

            p0 = 0
            for si, nk in enumerate(STEPS):
                psl = slice(p0, p0 + nk)
                last = si == len(STEPS) - 1
                # input DMA, one per ring; step 0 split so the k2 operands
                # (ar+an+bd = first 1024 elements) land first
                for r in range(2):
                    if si == 0:
                        ring[r].dma_start(it[r][:, psl, :BS0],
                                          dram[r][:, psl, :BS0])
                        ring[r].dma_start(it[r][:, psl, BS0:],
                                          dram[r][:, psl, BS0:])
                    else:
                        ring[r].dma_start(it[r][:, psl, :], dram[r][:, psl, :])
                # j1 operand prep: as = ar - an, bsub = bs - bd
                for r in range(2):
                    nc.vector.tensor_tensor(
                        as_t[r][:, psl, :], it[r][:, psl, AR0:AR0 + BATCH],
                        it[r][:, psl, AN0:AN0 + BATCH],
                        mybir.AluOpType.subtract)
                    nc.vector.tensor_tensor(
                        bsub_t[r][:, psl, :], it[r][:, psl, BS0:BS0 + NSH],
                        it[r][:, psl, BD0:BD0 + NSH],
                        mybir.AluOpType.subtract)
                if last:
                    # m-major so m0's banks finish ~12 matmuls early and
                    # its combine/store overlaps m1's final matmuls
                    for m in range(MT):
                        for pos in range(p0, p0 + nk):
                            for r in range(2):
                                emit_mms(2 * pos + r, m)
                else:
                    for pos in range(p0, p0 + nk):
                        for r in range(2):
                            kt = 2 * pos + r
                            # k2/k3 first (no prep dependency), j1 last
                            for m in range(MT):
                                emit_mms(kt, m, comps=("k2",))
                            for m in range(MT):
                                emit_mms(kt, m, comps=("k3",))
                            for m in range(MT):
                                emit_mms(kt, m, comps=("j1",))
                p0 += nk

            for m in range(MT):
                msl = slice(m * 128, (m + 1) * 128)
                t1 = o_pool.tile([128, NSH], DT, name=f"t1_{m}")
                out_r = o_pool.tile([128, NSH], DT, name=f"or_{m}")
                out_i = o_pool.tile([128, NSH], DT, name=f"oi_{m}")
                # t1 = 0.5*j1 on ACT (PSUM -> SBUF), adds on DVE
                nc.scalar.activation(t1[:], ps[(m, "j1")][:],
                                     mybir.ActivationFunctionType.Copy,
                                     scale=0.5)
                nc.vector.tensor_tensor(out_r[:], ps[(m, "k3")][:], t1[:],
                                        mybir.AluOpType.add)
                nc.vector.tensor_tensor(out_i[:], ps[(m, "k2")][:], t1[:],
                                        mybir.AluOpType.add)
                nc.sync.dma_start(c_r[msl, :], out_r[:])
                nc.scalar.dma_start(c_i[msl, :], out_i[:])

    nc.compile()
    return nc


def _get_nc(dt_name=DT_NAME):
    if dt_name not in _CACHE:
        _CACHE[dt_name] = _build(dt_name)
    return _CACHE[dt_name]


def _pack_core(ar, an, bd, bs):
    """ar/an: [4096, 256] f16, bd/bs: [4096, 512] f16 ->
    (in0, in1) each [128, 16, 1536] (even/odd k-tiles)."""
    full = np.concatenate([
        ar.reshape(KT, 128, BATCH),
        an.reshape(KT, 128, BATCH),
        bd.reshape(KT, 128, NSH),
        bs.reshape(KT, 128, NSH),
    ], axis=2)  # [KT, 128, ROW]
    in0 = np.ascontiguousarray(full[0::2].transpose(1, 0, 2))
    in1 = np.ascontiguousarray(full[1::2].transpose(1, 0, 2))
    return in0, in1


def run_device(A, B, dt_name=DT_NAME, trace=False):
    """A: [256, 4096] complex64, B: [4096, 4096] complex64.
    Returns C = A @ B as [256, 4096] complex64 plus the raw results."""
    from concourse import bass_utils

    nc = _get_nc(dt_name)

    at = A.T  # [4096, 256]
    ar = np.ascontiguousarray(at.real).astype(np.float16)
    an = np.ascontiguousarray(-at.imag).astype(np.float16)
    br_full = B.real
    bi_full = B.imag

    in_maps = []
    for c in range(N_CORES):
        csl = slice(c * NSH, (c + 1) * NSH)
        br = br_full[:, csl]
        bi = bi_full[:, csl]
        bd = (bi - br).astype(np.float16)
        bs = (br + bi).astype(np.float16)
        in0, in1 = _pack_core(ar, an, bd, bs)
        in_maps.append({"in0": in0, "in1": in1})

    res = bass_utils.run_bass_kernel_spmd(
        nc, in_maps, core_ids=list(range(N_CORES)), trace=trace
    )

    out = np.empty((BATCH, HALF), dtype=np.complex64)
    for c in range(N_CORES):
        csl = slice(c * NSH, (c + 1) * NSH)
        out.real[:, csl] = res.results[c]["c_r"]
        out.imag[:, csl] = res.results[c]["c_i"]
    return out, res


def kernel(state, target_matrix, control, num_qubits):
    state = np.asarray(state)
    target_matrix = np.asarray(target_matrix)
    control = int(control)
    num_qubits = int(num_qubits)
    dim = 1 << num_qubits

    assert state.shape == (BATCH, DIM) and dim == DIM, (
        "kernel hardcoded for [256, 8192]"
    )

    mask = 1 << (num_qubits - control - 1)
    idx = np.arange(dim)
    c1 = idx[(idx & mask) != 0]  # columns with control bit set

    if control == 0:
        A = state[:, HALF:]
        B = target_matrix[HALF:, HALF:]
    else:
        A = state[:, c1]
        B = target_matrix[np.ix_(c1, c1)]
    A = np.ascontiguousarray(A, dtype=np.complex64)
    B = np.ascontiguousarray(B, dtype=np.complex64)

    C, _ = run_device(A, B)

    out = state.astype(np.complex64, copy=True)
    out[:, c1] = C
    return out


if __name__ == "__main__":
    # quick numeric self-check against numpy on random data
    rng = np.random.default_rng(0)
    A = (rng.standard_normal((BATCH, HALF)) +
         1j * rng.standard_normal((BATCH, HALF))).astype(np.complex64) / 90.5
    B = (rng.standard_normal((HALF, HALF)) +
         1j * rng.standard_normal((HALF, HALF))).astype(np.complex64) / 90.5
    C, _ = run_device(A, B)
    ref = A @ B
    err = np.linalg.norm(C - ref) / np.linalg.norm(ref)
    print("rel err vs numpy:", err)
